# revision 5
# baseline (speedup 1.0000x reference)
"""Trainium2 Bass kernel for the anchor-based NMS matcher.

Math (see problem reference): per (batch b, organ o), over Qp=8192 anchor
queries q:
    cost_class = -sigmoid(logit)
    cost_bbox  = sum_d |anchor_d - tgt_d|            (cxcyczwhd space)
    cost_giou  = -giou3d(xyzxyz(clip(anchor,0)), xyzxyz(tgt))
    C = 5*cb + 2*cc + 2*cg
    matches     = one_hot(argmin_q C) * present
    soft_labels = present ? clip((cg-cgmax)/(cgmin-cgmax), 0) : -1

Device strategy (8 cores, data-parallel over batch, 2 batch items/core):
  SBUF layout: 120 partitions = (b_local 2) x (organ 20) x (q-chunk 3),
  free dim N=2752 (3*2752=8256, q padded 8192->8256 with edge dup).
  All per-(b,o) target quantities become per-partition scalars, enabling
  fused tensor_scalar / scalar_tensor_tensor / activation(bias,scale) ops.
  giou is reduced to one reciprocal via
      -giou + 1 = 1 - inter/union - union/vol_c = 1 - (u^2 + inter*vol_c)/(u*vol_c)
  and we rank with negC = sig - 2.5*cb + frac  (argmax negC == argmin C) and
  normalize soft labels directly in frac-space (affine-invariant).
  Per-partition argmax via DVE max/max_index; the 3 q-chunks per (b,o) are
  combined through tiny PE transposes ([120,1] <-> [1,120]) so all the
  cross-chunk logic runs on partition-0 row vectors.
"""

import numpy as np

import concourse.bacc as bacc
import concourse.bass as bass
import concourse.mybir as mybir
from concourse.bass_utils import run_bass_kernel_spmd
from concourse.masks import make_identity
from concourse.tile import TileContext

F32 = mybir.dt.float32
I32 = mybir.dt.int32
U32 = mybir.dt.uint32
ALU = mybir.AluOpType
ACTF = mybir.ActivationFunctionType
AXL = mybir.AxisListType

BS, O, QP = 16, 20, 8192
NCORES = 8
BL = BS // NCORES        # batch items per core
NCH = 3                  # q chunks per organ
N = 2752                 # chunk width; 3*2752 = 8256 = 8192 + 64 pad
NPAIR = BL * O           # 40 (b,o) pairs per core
P = NPAIR * NCH          # 120 partitions
NPLANES = 16             # a0..a5, alt0..2, arb0..2, rs0..2, vola

_BUILT = {}


def _build_nc():
    nc = bacc.Bacc("TRN2", target_bir_lowering=False, debug=False)
    ath = nc.dram_tensor("ath", [NPLANES, 60, N], F32, kind="ExternalInput")
    lg = nc.dram_tensor("lg", [P, N], F32, kind="ExternalInput")
    sc = nc.dram_tensor("sc", [P, 20], F32, kind="ExternalInput")
    rw = nc.dram_tensor("rw", [1, 384], F32, kind="ExternalInput")
    mout = nc.dram_tensor("mout", [P, N], I32, kind="ExternalOutput")
    sout = nc.dram_tensor("sout", [P, N], F32, kind="ExternalOutput")

    with TileContext(nc) as tc:
        with (
            tc.tile_pool(name="big", bufs=1) as big,
            tc.tile_pool(name="sm", bufs=1) as sm,
            tc.tile_pool(name="ps", bufs=1, space="PSUM") as ps,
        ):
            # ---------------- small/const tiles ----------------
            sct = sm.tile([P, 20], F32, tag="sct")
            nc.sync.dma_start(out=sct[:], in_=sc[:])
            rwt = sm.tile([1, 384], F32, tag="rwt")
            nc.sync.dma_start(out=rwt[:], in_=rw[:])
            ident = sm.tile([120, 120], F32, tag="ident")
            make_identity(nc, ident[:])
            ones11 = sm.tile([1, 1], F32, tag="ones11")
            nc.vector.memset(ones11[:], 1.0)

            def col(i):  # per-partition scalar column from sc
                return sct[:, i : i + 1]

            NT = [col(d) for d in range(6)]       # -t_d
            BLT = [col(6 + d) for d in range(3)]
            BRB = [col(9 + d) for d in range(3)]
            FD = [col(12 + d) for d in range(3)]
            VOLB = col(15)
            QOFF = col(16)
            FLOOR = col(17)
            qoff_row = rwt[:, 0:120]
            prs_row = rwt[:, 120:240]
            prsm1_row = rwt[:, 240:360]

            # ---------------- big plane inputs ----------------
            def load_plane(i, tag):
                t = big.tile([P, N], F32, tag=tag, name=tag)
                lo, hi = (0, 60) if i % 2 == 0 else (60, 120)
                nc.sync.dma_start(out=t[lo:hi, :], in_=ath[i])
                nc.sync.dma_start(out=t[120 - hi : 120 - lo, :], in_=t[lo:hi, :])
                return t

            a = [load_plane(d, f"a{d}") for d in range(6)]
            alt = [load_plane(6 + d, f"alt{d}") for d in range(3)]
            arb = [load_plane(9 + d, f"arb{d}") for d in range(3)]
            rs = [load_plane(12 + d, f"rs{d}") for d in range(3)]
            vola = load_plane(15, "vola")
            lgt = big.tile([P, N], F32, tag="lg")
            nc.sync.dma_start(out=lgt[:], in_=lg[:])

            # ---------------- big passes ----------------
            # sigmoid (in place over logits)
            nc.scalar.activation(lgt[:], lgt[:], ACTF.Sigmoid)
            sig = lgt

            # cost_bbox: ab_d = |a_d - t_d| in place over a_d (ACT)
            for d in range(6):
                nc.scalar.activation(a[d][:], a[d][:], ACTF.Abs, bias=NT[d], scale=1.0)
            # tree sum into a0
            nc.gpsimd.tensor_tensor(out=a[0][:], in0=a[0][:], in1=a[1][:], op=ALU.add)
            nc.vector.tensor_tensor(out=a[2][:], in0=a[2][:], in1=a[3][:], op=ALU.add)
            nc.gpsimd.tensor_tensor(out=a[4][:], in0=a[4][:], in1=a[5][:], op=ALU.add)
            nc.vector.tensor_tensor(out=a[0][:], in0=a[0][:], in1=a[2][:], op=ALU.add)
            nc.vector.tensor_tensor(out=a[0][:], in0=a[0][:], in1=a[4][:], op=ALU.add)
            cb = a[0]

            # giou: mx_d = max(alt_d, Blt_d) (in place over alt)
            for d in range(3):
                nc.vector.tensor_scalar_max(out=alt[d][:], in0=alt[d][:], scalar1=BLT[d])
            # m_d = min(arb_d, Brb_d) - mx_d (in place over arb)
            for d in range(3):
                nc.vector.scalar_tensor_tensor(
                    out=arb[d][:], in0=arb[d][:], scalar=BRB[d], in1=alt[d][:],
                    op0=ALU.min, op1=ALU.subtract,
                )
            m = arb
            # vc_d = (rs_d + f_d) - m_d (in place over rs)
            for d in range(3):
                nc.vector.scalar_tensor_tensor(
                    out=rs[d][:], in0=rs[d][:], scalar=FD[d], in1=m[d][:],
                    op0=ALU.add, op1=ALU.subtract,
                )
            vc = rs
            # r_d = relu(m_d) in place; inter = r0*r1*r2 into m0
            for d in range(3):
                nc.vector.tensor_scalar_max(out=m[d][:], in0=m[d][:], scalar1=0.0)
            nc.gpsimd.tensor_tensor(out=m[0][:], in0=m[0][:], in1=m[1][:], op=ALU.mult)
            nc.gpsimd.tensor_tensor(out=m[0][:], in0=m[0][:], in1=m[2][:], op=ALU.mult)
            inter = m[0]
            # vol_c into alt[2] slot (dead mx)
            volc = big.tile([P, N], F32, tag="alt2")
            nc.gpsimd.tensor_tensor(out=volc[:], in0=vc[0][:], in1=vc[1][:], op=ALU.mult)
            nc.gpsimd.tensor_tensor(out=volc[:], in0=volc[:], in1=vc[2][:], op=ALU.mult)
            # union = (vola + volb) - inter (in place over vola)
            nc.vector.scalar_tensor_tensor(
                out=vola[:], in0=vola[:], scalar=VOLB, in1=inter[:],
                op0=ALU.add, op1=ALU.subtract,
            )
            union = vola
            # u2 = union^2 into alt[1] slot
            u2 = big.tile([P, N], F32, tag="alt1")
            nc.scalar.activation(u2[:], union[:], ACTF.Square)
            # ivc = inter * volc (in place over inter == m0/arb0)
            nc.gpsimd.tensor_tensor(out=inter[:], in0=inter[:], in1=volc[:], op=ALU.mult)
            ivc = inter
            # den = union * volc (in place over volc)
            nc.vector.tensor_tensor(out=volc[:], in0=union[:], in1=volc[:], op=ALU.mult)
            den = volc
            # num = u2 + ivc (in place over u2)
            nc.vector.tensor_tensor(out=u2[:], in0=u2[:], in1=ivc[:], op=ALU.add)
            num = u2
            # rden ~= 1/den (2 ULP), scratch in dead arb0-chain slot
            rden = big.tile([P, N], F32, tag="alt0")
            scratch = big.tile([P, N], F32, tag="arb0")
            nc.vector.reciprocal_approx_accurate(out=rden[:], in_=den[:], scratch=scratch[:])
            # frac = num * rden (in place over num; frac = iou + union/volc - 1... kept to end)
            nc.vector.tensor_tensor(out=num[:], in0=num[:], in1=rden[:], op=ALU.mult)
            frac = num
            # negC/2 = (cb * -2.5 + sig) + frac   (in place: u1 over sig, negc over u1)
            nc.vector.scalar_tensor_tensor(
                out=sig[:], in0=cb[:], scalar=-2.5, in1=sig[:],
                op0=ALU.mult, op1=ALU.add,
            )
            nc.vector.tensor_tensor(out=sig[:], in0=sig[:], in1=frac[:], op=ALU.add)
            negc = sig

            # per-partition top-8 + index, frac min/max
            mx8 = sm.tile([P, 8], F32, tag="mx8")
            ix8 = sm.tile([P, 8], U32, tag="ix8")
            nc.vector.max(out=mx8[:], in_=negc[:])
            nc.vector.max_index(out=ix8[:], in_max=mx8[:], in_values=negc[:])
            fmx = sm.tile([P, 1], F32, tag="fmx")
            fmn = sm.tile([P, 1], F32, tag="fmn")
            nc.vector.tensor_reduce(out=fmx[:], in_=frac[:], axis=AXL.X, op=ALU.max)
            nc.vector.tensor_reduce(out=fmn[:], in_=frac[:], axis=AXL.X, op=ALU.min)

            # ---------------- cross-chunk combine (tiny) ----------------
            ixf = sm.tile([P, 1], F32, tag="ixf")
            nc.vector.tensor_copy(out=ixf[:], in_=ix8[:, 0:1])
            gidx = sm.tile([P, 1], F32, tag="gidx")
            nc.vector.tensor_scalar_add(out=gidx[:], in0=ixf[:], scalar1=QOFF)

            def to_row(colap, tag):  # [120,1] sbuf -> [1,120] psum
                r = ps.tile([1, 120], F32, tag=tag, name=tag)
                nc.tensor.transpose(r[:], colap, ident[:])
                return r

            mx_t = to_row(mx8[:, 0:1], "mx_t")
            gx_t = to_row(gidx[:], "gx_t")
            fx_t = to_row(fmx[:], "fx_t")
            fn_t = to_row(fmn[:], "fn_t")

            def g3(ap120):  # [1,120] -> [1,40,3]
                return ap120.rearrange("p (g c) -> p g c", c=3)

            def bcast3(dst120, src40):
                dr = g3(dst120)
                for k in range(3):
                    nc.vector.tensor_copy(out=dr[:, :, k], in_=src40)

            def srow(tag, w=120):
                return sm.tile([1, w], F32, tag=tag, name=tag)

            # group max of chunk maxima, winner-takes-first
            gmax = srow("gmax", 40)
            nc.vector.tensor_reduce(out=gmax[:], in_=g3(mx_t[:]), axis=AXL.X, op=ALU.max)
            gmax3 = srow("gmax3")
            bcast3(gmax3[:], gmax[:])
            eqm = srow("eqm")
            nc.vector.tensor_tensor(out=eqm[:], in0=mx_t[:], in1=gmax3[:], op=ALU.is_equal)
            eq = g3(eqm[:])
            ne0 = srow("ne0", 40)
            ne1 = srow("ne1", 40)
            nc.vector.tensor_scalar(out=ne0[:], in0=eq[:, :, 0], scalar1=-1.0,
                                    scalar2=1.0, op0=ALU.mult, op1=ALU.add)
            nc.vector.tensor_scalar(out=ne1[:], in0=eq[:, :, 1], scalar1=-1.0,
                                    scalar2=1.0, op0=ALU.mult, op1=ALU.add)
            w1 = srow("w1", 40)
            nc.vector.tensor_tensor(out=w1[:], in0=eq[:, :, 1], in1=ne0[:], op=ALU.mult)
            w2 = srow("w2", 40)
            nc.vector.tensor_tensor(out=w2[:], in0=eq[:, :, 2], in1=ne0[:], op=ALU.mult)
            nc.vector.tensor_tensor(out=w2[:], in0=w2[:], in1=ne1[:], op=ALU.mult)
            gx = g3(gx_t[:])
            t0 = srow("t0", 40)
            t1 = srow("t1", 40)
            t2 = srow("t2", 40)
            nc.vector.tensor_tensor(out=t0[:], in0=eq[:, :, 0], in1=gx[:, :, 0], op=ALU.mult)
            nc.vector.tensor_tensor(out=t1[:], in0=w1[:], in1=gx[:, :, 1], op=ALU.mult)
            nc.vector.tensor_tensor(out=t2[:], in0=w2[:], in1=gx[:, :, 2], op=ALU.mult)
            nc.vector.tensor_tensor(out=t0[:], in0=t0[:], in1=t1[:], op=ALU.add)
            nc.vector.tensor_tensor(out=t0[:], in0=t0[:], in1=t2[:], op=ALU.add)
            aw = t0  # [1,40] global argmin q (f32, exact int)
            aw3 = srow("aw3")
            bcast3(aw3[:], aw[:])
            # awl = (aw3 - qoff) * prs + (prs - 1)
            nc.vector.tensor_tensor(out=aw3[:], in0=aw3[:], in1=qoff_row, op=ALU.subtract)
            nc.vector.tensor_tensor(out=aw3[:], in0=aw3[:], in1=prs_row, op=ALU.mult)
            nc.vector.tensor_tensor(out=aw3[:], in0=aw3[:], in1=prsm1_row, op=ALU.add)

            # soft-label scale/bias rows
            gfx = srow("gfx", 40)
            gfn = srow("gfn", 40)
            nc.vector.tensor_reduce(out=gfx[:], in_=g3(fx_t[:]), axis=AXL.X, op=ALU.max)
            nc.vector.tensor_reduce(out=gfn[:], in_=g3(fn_t[:]), axis=AXL.X, op=ALU.min)
            dd = srow("dd", 40)
            nc.vector.tensor_tensor(out=dd[:], in0=gfx[:], in1=gfn[:], op=ALU.subtract)
            inv = srow("inv", 40)
            nc.vector.reciprocal(out=inv[:], in_=dd[:])
            nb = srow("nb", 40)
            nc.vector.tensor_tensor(out=nb[:], in0=gfn[:], in1=inv[:], op=ALU.mult)
            nc.vector.tensor_scalar_mul(out=nb[:], in0=nb[:], scalar1=-1.0)
            inv3 = srow("inv3")
            nb3 = srow("nb3")
            bcast3(inv3[:], inv[:])
            bcast3(nb3[:], nb[:])
            nc.vector.tensor_tensor(out=inv3[:], in0=inv3[:], in1=prs_row, op=ALU.mult)
            nc.vector.tensor_tensor(out=nb3[:], in0=nb3[:], in1=prs_row, op=ALU.mult)
            nc.vector.tensor_tensor(out=nb3[:], in0=nb3[:], in1=prsm1_row, op=ALU.add)

            def to_col(rowap, tag):  # [1,120] sbuf -> [120,1] psum
                c = ps.tile([120, 1], F32, tag=tag, name=tag)
                nc.tensor.transpose(c[:], rowap, ones11[:])
                return c

            caw = to_col(aw3[:], "caw")
            csc = to_col(inv3[:], "csc")
            cbi = to_col(nb3[:], "cbi")
            awl_f = sm.tile([P, 1], F32, tag="awl_f")
            nc.vector.tensor_copy(out=awl_f[:], in_=caw[:])
            scale_c = sm.tile([P, 1], F32, tag="scale_c")
            nc.vector.tensor_copy(out=scale_c[:], in_=csc[:])
            bias_c = sm.tile([P, 1], F32, tag="bias_c")
            nc.vector.tensor_copy(out=bias_c[:], in_=cbi[:])

            # ---------------- outputs ----------------
            iota_t = big.tile([P, N], F32, tag="a1")
            nc.gpsimd.iota(iota_t[:], pattern=[[1, N]], channel_multiplier=0,
                           allow_small_or_imprecise_dtypes=True)
            mt = big.tile([P, N], I32, tag="a3")
            nc.vector.tensor_scalar(out=mt[:], in0=iota_t[:], scalar1=awl_f[:],
                                    scalar2=None, op0=ALU.is_equal)
            nc.sync.dma_start(out=mout[:], in_=mt[:])
            slt = big.tile([P, N], F32, tag="a5")
            nc.scalar.activation(slt[:], frac[:], ACTF.Identity,
                                 bias=bias_c[:], scale=scale_c[:])
            nc.vector.tensor_scalar_max(out=slt[:], in0=slt[:], scalar1=FLOOR)
            nc.sync.dma_start(out=sout[:], in_=slt[:])

    nc.finalize()
    return nc


def _prep_host(pred_logits, anchors, target_boxes, target_present):
    f32 = np.float32
    A = np.ascontiguousarray(anchors.reshape(O, QP, 6).astype(f32, copy=False))
    pad = lambda x: np.pad(x, ((0, 0), (0, NCH * N - QP)), mode="edge")

    planes = []
    comp = [pad(A[:, :, d]) for d in range(6)]  # [20, 8256] each
    planes += comp
    rc = [np.maximum(comp[d], f32(0)) for d in range(3)]
    rsz = [np.maximum(comp[3 + d], f32(0)) for d in range(3)]
    alt = [rc[d] - f32(0.5) * rsz[d] for d in range(3)]
    arb = [rc[d] + f32(0.5) * rsz[d] for d in range(3)]
    planes += alt + arb + rsz
    planes.append((rsz[0] * rsz[1]) * rsz[2])
    ath = np.stack([p.reshape(O, NCH, N).reshape(60, N) for p in planes])
    ath = np.ascontiguousarray(ath, dtype=f32)

    lgs = pred_logits.reshape(BS, O, QP).astype(f32, copy=False)
    lgs = np.pad(lgs, ((0, 0), (0, 0), (0, NCH * N - QP)), mode="edge")
    lg_all = lgs.reshape(BS, O, NCH, N)

    t = target_boxes.astype(f32, copy=False)          # [BS, O, 6]
    tc_, ts_ = t[..., :3], t[..., 3:]
    blt = tc_ - f32(0.5) * ts_
    brb = tc_ + f32(0.5) * ts_
    fd = brb - blt
    volb = (fd[..., 0] * fd[..., 1]) * fd[..., 2]
    prs = target_present.astype(f32, copy=False)      # [BS, O]

    in_maps = []
    for c in range(NCORES):
        b0 = c * BL
        lg = np.ascontiguousarray(
            lg_all[b0 : b0 + BL].reshape(P, N), dtype=f32)
        sc = np.zeros((P, 20), f32)
        sc3 = sc.reshape(BL * O, NCH, 20)
        tb = t[b0 : b0 + BL].reshape(BL * O, 6)
        sc3[:, :, 0:6] = -tb[:, None, :]
        sc3[:, :, 6:9] = blt[b0 : b0 + BL].reshape(-1, 3)[:, None, :]
        sc3[:, :, 9:12] = brb[b0 : b0 + BL].reshape(-1, 3)[:, None, :]
        sc3[:, :, 12:15] = fd[b0 : b0 + BL].reshape(-1, 3)[:, None, :]
        sc3[:, :, 15] = volb[b0 : b0 + BL].reshape(-1)[:, None]
        sc3[:, :, 16] = np.arange(NCH, dtype=f32)[None, :] * f32(N)
        pr = prs[b0 : b0 + BL].reshape(-1)
        sc3[:, :, 17] = (pr[:, None] - f32(1))  # floor: 0 if present else -1
        rwv = np.zeros((1, 384), f32)
        qoff = np.tile(np.arange(NCH, dtype=f32) * f32(N), BL * O)
        rwv[0, 0:P] = qoff
        pr3 = np.repeat(pr, NCH)
        rwv[0, 120 : 120 + P] = pr3
        rwv[0, 240 : 240 + P] = pr3 - f32(1)
        in_maps.append({"ath": ath, "lg": lg, "sc": sc, "rw": rwv})
    return in_maps


def kernel(pred_logits, pred_boxes, anchors, target_boxes, target_present,
           num_top_queries):
    k = int(num_top_queries)
    assert k == 1, f"kernel specialized for num_top_queries=1, got {k}"

    key = "nc"
    if key not in _BUILT:
        _BUILT[key] = _build_nc()
    nc = _BUILT[key]

    in_maps = _prep_host(np.asarray(pred_logits), np.asarray(anchors),
                         np.asarray(target_boxes), np.asarray(target_present))
    res = run_bass_kernel_spmd(nc, in_maps, core_ids=list(range(NCORES)))

    matches = np.empty((BS, O, QP), np.int32)
    soft = np.empty((BS, O, QP), np.float32)
    for c, r in enumerate(res.results):
        b0 = c * BL
        m = r["mout"].reshape(BL, O, NCH * N)[:, :, :QP]
        s = r["sout"].reshape(BL, O, NCH * N)[:, :, :QP]
        matches[b0 : b0 + BL] = m
        soft[b0 : b0 + BL] = s
    return matches, soft
